# revision 18
# baseline (speedup 1.0000x reference)
"""Trainium2 Bass kernel for nn_Loss_1_8323646620405 (multi-head BCE/CCE loss).

Data-parallel over batch: 8 cores x 8 batches. Host re-encodes inputs
losslessly (f32->bf16 planar planes; target bits packed into two int16
mask planes). Device computes three ACT-accumulated ln() streams:

  A1 = sum ln(s ? ps : 1-ps)          [stroke BCE, unweighted part]
  A2 = sum s*ln(ps)                   [stroke BCE, (W1-W0) part]
  A3 = sum s*ln(pe*he*pt*sv)          [player/hand BCE + point/serve CCE]

host: loss = -(W0*A1 + (W1-W0)*A2 + A3) / (B*S)

Two custom DVE ops (select-style) fuse the BCE folds and the s-gating;
one-hot-gated sum-form selects compute pt/sv, split across DVE and the
GPSIMD (Pool) engine to balance the three compute engines.
"""

import numpy as np

import concourse.bass as bass
import concourse.mybir as mybir
import concourse.tile as tile
from concourse.bass_utils import run_bass_kernel_spmd

# ---- walrus single-wait workaround ----------------------------------------
# This container's walrus build encodes at most ONE semaphore wait per
# instruction ('Too many sync wait commands'). Tile's scheduler freely
# attaches N waits to one instruction. Two patches:
#  1. postorder_instruction_blocks wrapper: split any instruction carrying
#     >1 wait -- extra waits move to same-engine NoOps inserted before it.
#  2. _drain_and_barrier: one drain per outstanding logical processor.
import bass_rust
from concourse.tile_cfg import postorder_instruction_blocks as _orig_post

_DMA_PROC_START = 10  # Collectives/DMASW*/DMAHW* procs inc by 16 per tick
_nop_ctr = [0]


def _split_waits_in_list(insts):
    out = []
    for ins in insts:
        si = getattr(ins, "sync_info", None)
        waits = list(si.on_wait) if si is not None else []
        if len(waits) > 1:
            for w in waits[:-1]:
                _nop_ctr[0] += 1
                nop = mybir.InstNoOp(name=f"WSPL-{_nop_ctr[0]}", ins=[], outs=[])
                nop.engine = ins.engine
                nop.sync_info = bass_rust.SyncInfo(on_wait=[w], on_update=[])
                out.append(nop)
            ins.sync_info = bass_rust.SyncInfo(
                on_wait=[waits[-1]], on_update=list(si.on_update)
            )
        out.append(ins)
    return out


def _patched_post(instructions, start_bb, output):
    for k in list(instructions.keys()):
        instructions[k] = _split_waits_in_list(instructions[k])
    return _orig_post(instructions, start_bb, output)


def _split_drain_and_barrier(self, tick_clock, wait_clock):
    gc = tick_clock.global_clock
    alloc = wait_clock.sems.allocated()
    for proc in sorted(alloc):
        tick = gc.peek_next(proc) - 1
        if tick <= 0:
            continue
        scale = 16 if proc >= _DMA_PROC_START else 1
        d = self.nc.sync.drain()
        d.wait_op(alloc[proc], tick * scale, "sem-ge")

    self.nc.all_engine_barrier()
    popped = self.nc._tile_sem_poison_stack.pop()
    assert popped is self._sem_poison
    self.nc.clear_and_free_semaphores(list(self.sems.allocated().values()))
    self.nc.all_engine_barrier()


tile.postorder_instruction_blocks = _patched_post
tile.TileContext._drain_and_barrier = _split_drain_and_barrier

# ---- custom DVE ops --------------------------------------------------------
# SEL_PROB_ANT:  out = cond ? a : 1-a        (BCE effective-probability fold)
# SEL_M1Z_ANT:   out = cond ? a-1 : 0        (s-gated ln via Ln(out+1))
# Registered at import, idempotently; shas self-pinned via the same
# lower() path compile() uses.
from concourse.dve_ops import (
    OPS as _DVE_OPS,
    _CUSTOM_DVE_ROW_BASE,
    _SUB_OPCODE_FOR_NAME,
    CUSTOM_DVE_SPECS,
    DveOp,
)
from concourse.dve_spec import Spec, Src0, Src1, One, Zero, select, lower as _dve_lower
from concourse.dve_uop import DveOpSpec


def _register_dve_op(name, spec):
    for op in _DVE_OPS:
        if op.name == name:
            return op
    row = _CUSTOM_DVE_ROW_BASE + len(_DVE_OPS)
    shas = {}
    for ver in ("v3", "v4"):
        s = DveOpSpec(name=name, opcode=row, uops=_dve_lower(spec, ver=ver), rd1_en=True)
        shas[ver] = s.sha(ver)
    op = DveOp(name, spec, subdim=False, uops_sha=shas)
    _DVE_OPS.append(op)
    _SUB_OPCODE_FOR_NAME[name] = row
    CUSTOM_DVE_SPECS[name] = spec
    return op


SEL_PROB = _register_dve_op(
    "SEL_PROB_ANT",
    Spec(
        body=select(Src1, Src0, One - Src0),
        reference=lambda in0, in1, s0, s1, imm2: np.where(
            np.asarray(in1) != 0, np.asarray(in0, np.float32), 1.0 - np.asarray(in0, np.float32)
        ).astype(np.float32),
    ),
)

def _gate_sum_ref(in0, in1, s0, s1, imm2):
    b = np.where(np.asarray(in1) != 0, np.asarray(in0, np.float32), 0.0).astype(
        np.float32
    )
    return b, b.reshape(b.shape[0], -1).sum(axis=-1, keepdims=True)


from operator import add as _op_add

SEL_GSUM = _register_dve_op(
    "SEL_GSUM_ANT",
    Spec(
        body=select(Src1, Src0, Zero),
        accum=_op_add,
        reference=_gate_sum_ref,
    ),
)

# ---- problem constants -----------------------------------------------------
B, S, F = 64, 32768, 9
W0, W1 = 0.51, 19.05
C2 = W1 - W0

NCORES = 8
B_LOC = B // NCORES          # 8 batches per core
N = B_LOC * S                # 262144 elements per core
P = 128                      # SBUF partitions
FD = N // P                  # 2048 free-dim elements per partition
NPL = 12                     # planes: ps pp ph P0 P1 P2 Q0 Q1 Q2 Q3 m1 m2
# small head/tail minis: short DMA ramp and short drain tail; the packed
# block keeps DMA descriptors >= 512B even for tiny chunks
CHUNKS = [(0, 128), (128, 256), (384, 256), (640, 256), (896, 256), (1152, 256), (1408, 256), (1664, 256), (1920, 128)]
K = len(CHUNKS)

f32 = mybir.dt.float32
bf16 = mybir.dt.bfloat16
i16 = mybir.dt.int16
Alu = mybir.AluOpType
Act = mybir.ActivationFunctionType


def _build_nc() -> bass.Bass:
    nc = bass.Bass()

    # One packed input block per core: per chunk, per partition, NPL planes
    # of sz 2-byte elements contiguous -> one DMA per chunk, 128 descriptors
    # of NPL*sz*2 bytes each (bandwidth-cap throughput).
    blk_d = nc.declare_dram_parameter("blk", [NPL * N], i16, isOutput=False)
    acc_d = nc.declare_dram_parameter("acc", [P, 3 * K], f32, isOutput=True)

    # const AP for Ln bias=0.5 (same pattern as Bass.__init__ consts)
    c05 = nc.alloc_sbuf_tensor("const-float32-0.5", [P, 1], f32)
    nc.gpsimd.memset(c05.ap(), 0.5)
    nc.const_aps.aps[(f32, 0.5)] = c05.ap()
    nc.all_engine_barrier()

    with tile.TileContext(nc) as tc:
        with (
            tc.tile_pool(name="io", bufs=4) as io,
            tc.tile_pool(name="dec", bufs=3) as dc,
            tc.tile_pool(name="sel", bufs=3) as sp,
            tc.tile_pool(name="acc", bufs=1) as ac,
        ):
            accT = ac.tile([P, 3 * K], f32)
            st = [dict() for _ in range(K)]

            def stage_a(k):
                off, sz = CHUNKS[k]
                BLK = io.tile([P, NPL, sz], i16, tag="BLK")
                src = blk_d[NPL * P * off : NPL * P * (off + sz)].rearrange(
                    "(p t c) -> p t c", p=P, t=NPL
                )
                nc.sync.dma_start(BLK[:], src)
                st[k]["BLK"] = BLK

            def stage_b(k):
                off, sz = CHUNKS[k]
                BLK = st[k]["BLK"]
                ps = BLK[:, 0, :].bitcast(bf16)
                A2v = BLK[:, 1:3, :].bitcast(bf16)    # pp || ph
                PQa = BLK[:, 3:7:3, :].bitcast(bf16)  # P0 || Q0
                PQb = BLK[:, 4:8:3, :].bitcast(bf16)  # P1 || Q1
                PQc = BLK[:, 5:9:3, :].bitcast(bf16)  # P2 || Q2
                Q3 = BLK[:, 9, :].bitcast(bf16)
                m1 = BLK[:, 10, :]
                M2 = BLK[:, 10:12, :]
                m2 = BLK[:, 11, :]

                SGN = dc.tile([P, 2, sz], i16, tag="SGN")
                MS = dc.tile([P, sz], i16, tag="MS")
                G1 = dc.tile([P, 2, sz], bf16, tag="G1")
                G2 = dc.tile([P, 2, sz], bf16, tag="G2")
                G3 = dc.tile([P, 2, sz], bf16, tag="G3")
                GE = dc.tile([P, sz], bf16, tag="GE")
                EA = dc.tile([P, 2, sz], bf16, tag="EA")
                XA = dc.tile([P, 2, sz], i16, tag="XA")
                PH = dc.tile([P, 2, sz], bf16, tag="PH")
                ES = dc.tile([P, sz], bf16, tag="ES")
                XS = dc.tile([P, sz], i16, tag="XS")
                A1o = dc.tile([P, sz], bf16, tag="A1o")
                LPS = dc.tile([P, sz], bf16, tag="LPS")
                T1 = sp.tile([P, 2, sz], bf16, tag="T1")
                T2 = sp.tile([P, 2, sz], bf16, tag="T2")
                T3 = sp.tile([P, 2, sz], bf16, tag="T3")
                T4 = sp.tile([P, sz], bf16, tag="T4")
                S1 = sp.tile([P, 2, sz], bf16, tag="S1")
                S2 = sp.tile([P, 2, sz], bf16, tag="S2")
                Z1 = sp.tile([P, sz], bf16, tag="Z1")
                st[k].update(LPS=LPS, T4=T4, S2=S2, Z1=Z1, m1=m1)

                # --- decode (DVE tensor_scalar, 4x on 2-byte dtypes) ---
                # m1 bits: 0..8 = y0..y8 ; m2 bits: 0=y7, 4=y2, 5=y3, 6=y6
                # SGN: 0x8000 where label bit set (sign-flip masks for pp||ph)
                nc.vector.tensor_scalar(SGN[:], M2, 1, 15, Alu.bitwise_and, Alu.logical_shift_left)
                # MS: 0x8000 where s==0 (sign-flip for stroke: 1-ps arm)
                nc.vector.tensor_scalar(MS[:], m1, 0, 15, Alu.is_equal, Alu.logical_shift_left)
                nc.vector.tensor_scalar(G1[:], M2, 16, 0, Alu.bitwise_and, Alu.is_gt)
                nc.vector.tensor_scalar(G2[:], M2, 48, 32, Alu.bitwise_and, Alu.is_equal)
                nc.vector.tensor_scalar(G3[:, 0, :], m1, 48, 0, Alu.bitwise_and, Alu.is_equal)
                nc.vector.tensor_scalar(G3[:, 1, :], m2, 112, 64, Alu.bitwise_and, Alu.is_equal)
                nc.vector.tensor_scalar(GE[:], m2, 112, 0, Alu.bitwise_and, Alu.is_equal)

                # --- BCE folds: sign-flip (p-0.5) by label bit, then +0.5 ---
                nc.scalar.activation(EA[:], A2v, Act.Copy, bias=-0.5)
                nc.vector.tensor_tensor(XA[:], EA[:].bitcast(i16), SGN[:], op=Alu.bitwise_xor)
                nc.vector.tensor_scalar(PH[:], XA[:].bitcast(bf16), 0.5, 1.0, Alu.add, Alu.mult)
                nc.scalar.activation(ES[:], ps, Act.Copy, bias=-0.5)
                nc.vector.tensor_tensor(XS[:], ES[:].bitcast(i16), MS[:], op=Alu.bitwise_xor)
                # A1 += ln(ps_eff) ; ps_eff = (+-(ps-0.5)) + 0.5
                nc.scalar.activation(A1o[:], XS[:].bitcast(bf16), Act.Ln, bias=0.5,
                                     accum_out=accT[:, 3 * k : 3 * k + 1])
                nc.scalar.activation(LPS[:], ps, Act.Ln)

                # --- point/serve one-hot gated sums (Pool engine) ---
                nc.gpsimd.tensor_tensor(T1[:], G1[:], PQa, op=Alu.mult)
                nc.gpsimd.tensor_tensor(T2[:], G2[:], PQb, op=Alu.mult)
                nc.gpsimd.tensor_tensor(T3[:], G3[:], PQc, op=Alu.mult)
                nc.gpsimd.tensor_tensor(S1[:], T1[:], T2[:], op=Alu.add)
                nc.gpsimd.tensor_tensor(S2[:], S1[:], T3[:], op=Alu.add)
                # T4, Z1 on DVE (balance)
                nc.vector.tensor_tensor(T4[:], GE[:], Q3, op=Alu.mult)
                nc.vector.tensor_tensor(Z1[:], PH[:, 0, :], PH[:, 1, :], op=Alu.mult)

            def stage_c(k):
                off, sz = CHUNKS[k]
                s = st[k]
                SV = sp.tile([P, sz], bf16, tag="SV")
                Z2 = sp.tile([P, sz], bf16, tag="Z2")
                Z = sp.tile([P, sz], bf16, tag="Z")
                LZ = sp.tile([P, sz], bf16, tag="LZ")
                XPS = sp.tile([P, sz], bf16, tag="XPS")
                s.update(Z=Z, LZ=LZ)
                # A2 += s*ln(ps)  (gated sum of LPS)
                nc.vector._custom_dve(
                    SEL_GSUM, out=XPS[:], in0=s["LPS"][:], in1=s["m1"],
                    accum_out=accT[:, 3 * k + 1 : 3 * k + 2],
                )
                nc.gpsimd.tensor_tensor(SV[:], s["S2"][:, 1, :], s["T4"][:], op=Alu.add)
                nc.gpsimd.tensor_tensor(Z2[:], s["S2"][:, 0, :], SV[:], op=Alu.mult)
                nc.vector.tensor_tensor(Z[:], s["Z1"][:], Z2[:], op=Alu.mult)
                nc.scalar.activation(LZ[:], Z[:], Act.Ln)

            def stage_d(k):
                off, sz = CHUNKS[k]
                s = st[k]
                X3 = sp.tile([P, sz], bf16, tag="X3")
                # A3 += s*ln(Z)
                nc.vector._custom_dve(
                    SEL_GSUM, out=X3[:], in0=s["LZ"][:], in1=s["m1"],
                    accum_out=accT[:, 3 * k + 2 : 3 * k + 3],
                )

            # software pipeline: A(k) ... B(k) C(k-1) D(k-2)
            stage_a(0)
            stage_a(1)
            for k in range(K):
                if k + 2 < K:
                    stage_a(k + 2)
                stage_b(k)
                if k >= 1:
                    stage_c(k - 1)
                if k >= 2:
                    stage_d(k - 2)
            stage_c(K - 1)
            stage_d(K - 2)
            stage_d(K - 1)

            nc.sync.dma_start(acc_d[:], accT[:])

    return nc


_NC_CACHE = None


def _get_nc():
    global _NC_CACHE
    if _NC_CACHE is None:
        _NC_CACHE = _build_nc()
    return _NC_CACHE


def _to_bf16(x):
    import ml_dtypes

    return np.asarray(x, dtype=np.float32).astype(ml_dtypes.bfloat16)


def _pack_core(inputs, core):
    sl = slice(core * B_LOC, (core + 1) * B_LOC)
    planes = [
        inputs["y_pred_stroke"][sl, :, 0],
        inputs["y_pred_player"][sl, :, 0],
        inputs["y_pred_hand"][sl, :, 0],
        inputs["y_pred_point"][sl, :, 0],
        inputs["y_pred_point"][sl, :, 1],
        inputs["y_pred_point"][sl, :, 2],
        inputs["y_pred_serve"][sl, :, 0],
        inputs["y_pred_serve"][sl, :, 1],
        inputs["y_pred_serve"][sl, :, 2],
        inputs["y_pred_serve"][sl, :, 3],
    ]
    pl = np.empty((NPL, P, FD), dtype=np.uint16)
    for i, p in enumerate(planes):
        pl[i] = _to_bf16(p).reshape(P, FD).view(np.uint16)

    y = np.asarray(inputs["y_target"][sl], dtype=np.uint16)  # [B_LOC, S, 9] of 0/1
    yr = y.reshape(N, F)
    # m1: bits 0..8 = y0..y8 (9-bit bitmask; m1 != 0  <=>  s = any(y))
    w1 = (1 << np.arange(F, dtype=np.uint16)).astype(np.uint16)
    m1 = (yr * w1).sum(axis=1, dtype=np.uint16)
    # m2: bit re-placements for paired decode: 0=y7, 4=y2, 5=y3, 6=y6
    m2 = (
        yr[:, 7]
        | (yr[:, 2] << 4)
        | (yr[:, 3] << 5)
        | (yr[:, 6] << 6)
    ).astype(np.uint16)
    pl[10] = m1.reshape(P, FD)
    pl[11] = m2.reshape(P, FD)

    # chunk-major packed block: [chunk][p][plane][c] contiguous
    parts = []
    for off, sz in CHUNKS:
        parts.append(pl[:, :, off : off + sz].transpose(1, 0, 2).reshape(-1))
    blk = np.concatenate(parts).view(np.int16)
    return {"blk": blk}


def _shard_inputs(inputs):
    return [_pack_core(inputs, i) for i in range(NCORES)]


def kernel(**inputs) -> np.ndarray:
    nc = _get_nc()
    in_maps = _shard_inputs(inputs)
    res = run_bass_kernel_spmd(nc, in_maps, list(range(NCORES)))
    a1 = a2 = a3 = 0.0
    for r in res.results:
        a = r["acc"].astype(np.float64).reshape(P, K, 3)
        a1 += a[:, :, 0].sum()
        a2 += a[:, :, 1].sum()
        a3 += a[:, :, 2].sum()
    mean = -(W0 * a1 + C2 * a2 + a3) / float(B * S)
    return np.array([mean], dtype=np.float32)


# revision 20
# speedup vs baseline: 1.0308x; 1.0308x over previous
"""Trainium2 Bass kernel for nn_Loss_1_8323646620405 (multi-head BCE/CCE loss).

Data-parallel over batch: 8 cores x 8 batches. Host re-encodes inputs
losslessly (f32->bf16 planar planes; target bits packed into two int16
mask planes). Device computes three ACT-accumulated ln() streams:

  A1 = sum ln(s ? ps : 1-ps)          [stroke BCE, unweighted part]
  A2 = sum s*ln(ps)                   [stroke BCE, (W1-W0) part]
  A3 = sum s*ln(pe*he*pt*sv)          [player/hand BCE + point/serve CCE]

host: loss = -(W0*A1 + (W1-W0)*A2 + A3) / (B*S)

Two custom DVE ops (select-style) fuse the BCE folds and the s-gating;
one-hot-gated sum-form selects compute pt/sv, split across DVE and the
GPSIMD (Pool) engine to balance the three compute engines.
"""

import numpy as np

import concourse.bass as bass
import concourse.mybir as mybir
import concourse.tile as tile
from concourse.bass_utils import run_bass_kernel_spmd

# ---- walrus single-wait workaround ----------------------------------------
# This container's walrus build encodes at most ONE semaphore wait per
# instruction ('Too many sync wait commands'). Tile's scheduler freely
# attaches N waits to one instruction. Two patches:
#  1. postorder_instruction_blocks wrapper: split any instruction carrying
#     >1 wait -- extra waits move to same-engine NoOps inserted before it.
#  2. _drain_and_barrier: one drain per outstanding logical processor.
import bass_rust
from concourse.tile_cfg import postorder_instruction_blocks as _orig_post

_DMA_PROC_START = 10  # Collectives/DMASW*/DMAHW* procs inc by 16 per tick
_nop_ctr = [0]


def _split_waits_in_list(insts):
    out = []
    for ins in insts:
        si = getattr(ins, "sync_info", None)
        waits = list(si.on_wait) if si is not None else []
        if len(waits) > 1:
            for w in waits[:-1]:
                _nop_ctr[0] += 1
                nop = mybir.InstNoOp(name=f"WSPL-{_nop_ctr[0]}", ins=[], outs=[])
                nop.engine = ins.engine
                nop.sync_info = bass_rust.SyncInfo(on_wait=[w], on_update=[])
                out.append(nop)
            ins.sync_info = bass_rust.SyncInfo(
                on_wait=[waits[-1]], on_update=list(si.on_update)
            )
        out.append(ins)
    return out


def _patched_post(instructions, start_bb, output):
    for k in list(instructions.keys()):
        instructions[k] = _split_waits_in_list(instructions[k])
    return _orig_post(instructions, start_bb, output)


def _split_drain_and_barrier(self, tick_clock, wait_clock):
    gc = tick_clock.global_clock
    alloc = wait_clock.sems.allocated()
    for proc in sorted(alloc):
        tick = gc.peek_next(proc) - 1
        if tick <= 0:
            continue
        scale = 16 if proc >= _DMA_PROC_START else 1
        d = self.nc.sync.drain()
        d.wait_op(alloc[proc], tick * scale, "sem-ge")

    self.nc.all_engine_barrier()
    popped = self.nc._tile_sem_poison_stack.pop()
    assert popped is self._sem_poison
    self.nc.clear_and_free_semaphores(list(self.sems.allocated().values()))
    self.nc.all_engine_barrier()


tile.postorder_instruction_blocks = _patched_post
tile.TileContext._drain_and_barrier = _split_drain_and_barrier

# ---- custom DVE ops --------------------------------------------------------
# SEL_PROB_ANT:  out = cond ? a : 1-a        (BCE effective-probability fold)
# SEL_M1Z_ANT:   out = cond ? a-1 : 0        (s-gated ln via Ln(out+1))
# Registered at import, idempotently; shas self-pinned via the same
# lower() path compile() uses.
from concourse.dve_ops import (
    OPS as _DVE_OPS,
    _CUSTOM_DVE_ROW_BASE,
    _SUB_OPCODE_FOR_NAME,
    CUSTOM_DVE_SPECS,
    DveOp,
)
from concourse.dve_spec import Spec, Src0, Src1, One, Zero, select, lower as _dve_lower
from concourse.dve_uop import DveOpSpec


def _register_dve_op(name, spec):
    for op in _DVE_OPS:
        if op.name == name:
            return op
    row = _CUSTOM_DVE_ROW_BASE + len(_DVE_OPS)
    shas = {}
    for ver in ("v3", "v4"):
        s = DveOpSpec(name=name, opcode=row, uops=_dve_lower(spec, ver=ver), rd1_en=True)
        shas[ver] = s.sha(ver)
    op = DveOp(name, spec, subdim=False, uops_sha=shas)
    _DVE_OPS.append(op)
    _SUB_OPCODE_FOR_NAME[name] = row
    CUSTOM_DVE_SPECS[name] = spec
    return op


SEL_PROB = _register_dve_op(
    "SEL_PROB_ANT",
    Spec(
        body=select(Src1, Src0, One - Src0),
        reference=lambda in0, in1, s0, s1, imm2: np.where(
            np.asarray(in1) != 0, np.asarray(in0, np.float32), 1.0 - np.asarray(in0, np.float32)
        ).astype(np.float32),
    ),
)

def _gate_sum_ref(in0, in1, s0, s1, imm2):
    b = np.where(np.asarray(in1) != 0, np.asarray(in0, np.float32), 0.0).astype(
        np.float32
    )
    return b, b.reshape(b.shape[0], -1).sum(axis=-1, keepdims=True)


from operator import add as _op_add

SEL_GSUM = _register_dve_op(
    "SEL_GSUM_ANT",
    Spec(
        body=select(Src1, Src0, Zero),
        accum=_op_add,
        reference=_gate_sum_ref,
    ),
)

# ---- problem constants -----------------------------------------------------
B, S, F = 64, 32768, 9
W0, W1 = 0.51, 19.05
C2 = W1 - W0

NCORES = 8
B_LOC = B // NCORES          # 8 batches per core
N = B_LOC * S                # 262144 elements per core
P = 128                      # SBUF partitions
FD = N // P                  # 2048 free-dim elements per partition
NPL = 12                     # planes: ps pp ph P0 P1 P2 Q0 Q1 Q2 Q3 m1 m2
# small head/tail minis: short DMA ramp and short drain tail; the packed
# block keeps DMA descriptors >= 512B even for tiny chunks
CHUNKS = [(0, 128), (128, 256), (384, 384), (768, 384), (1152, 384), (1536, 384), (1920, 128)]
K = len(CHUNKS)

f32 = mybir.dt.float32
bf16 = mybir.dt.bfloat16
i16 = mybir.dt.int16
Alu = mybir.AluOpType
Act = mybir.ActivationFunctionType


def _build_nc() -> bass.Bass:
    nc = bass.Bass()

    # One packed input block per core: per chunk, per partition, NPL planes
    # of sz 2-byte elements contiguous -> one DMA per chunk, 128 descriptors
    # of NPL*sz*2 bytes each (bandwidth-cap throughput).
    blk_d = nc.declare_dram_parameter("blk", [NPL * N], i16, isOutput=False)
    acc_d = nc.declare_dram_parameter("acc", [P, 3 * K], f32, isOutput=True)

    # const AP for Ln bias=0.5 (same pattern as Bass.__init__ consts)
    c05 = nc.alloc_sbuf_tensor("const-float32-0.5", [P, 1], f32)
    nc.gpsimd.memset(c05.ap(), 0.5)
    nc.const_aps.aps[(f32, 0.5)] = c05.ap()
    nc.all_engine_barrier()

    with tile.TileContext(nc) as tc:
        with (
            tc.tile_pool(name="io", bufs=4) as io,
            tc.tile_pool(name="dec", bufs=3) as dc,
            tc.tile_pool(name="sel", bufs=3) as sp,
            tc.tile_pool(name="acc", bufs=1) as ac,
        ):
            accT = ac.tile([P, 3 * K], f32)
            st = [dict() for _ in range(K)]

            def stage_a(k):
                off, sz = CHUNKS[k]
                BLK = io.tile([P, NPL, sz], i16, tag="BLK")
                src = blk_d[NPL * P * off : NPL * P * (off + sz)].rearrange(
                    "(p t c) -> p t c", p=P, t=NPL
                )
                nc.sync.dma_start(BLK[:], src)
                st[k]["BLK"] = BLK

            def stage_b(k):
                off, sz = CHUNKS[k]
                BLK = st[k]["BLK"]
                ps = BLK[:, 0, :].bitcast(bf16)
                A2v = BLK[:, 1:3, :].bitcast(bf16)    # pp || ph
                PQa = BLK[:, 3:7:3, :].bitcast(bf16)  # P0 || Q0
                PQb = BLK[:, 4:8:3, :].bitcast(bf16)  # P1 || Q1
                PQc = BLK[:, 5:9:3, :].bitcast(bf16)  # P2 || Q2
                Q3 = BLK[:, 9, :].bitcast(bf16)
                m1 = BLK[:, 10, :]
                M2 = BLK[:, 10:12, :]
                m2 = BLK[:, 11, :]

                SGN = dc.tile([P, 2, sz], i16, tag="SGN")
                MS = dc.tile([P, sz], i16, tag="MS")
                G1 = dc.tile([P, 2, sz], bf16, tag="G1")
                G2 = dc.tile([P, 2, sz], bf16, tag="G2")
                G3 = dc.tile([P, 2, sz], bf16, tag="G3")
                GE = dc.tile([P, sz], bf16, tag="GE")
                EA = dc.tile([P, 2, sz], bf16, tag="EA")
                XA = dc.tile([P, 2, sz], i16, tag="XA")
                PH = dc.tile([P, 2, sz], bf16, tag="PH")
                ES = dc.tile([P, sz], bf16, tag="ES")
                XS = dc.tile([P, sz], i16, tag="XS")
                A1o = dc.tile([P, sz], bf16, tag="A1o")
                LPS = dc.tile([P, sz], bf16, tag="LPS")
                T1 = sp.tile([P, 2, sz], bf16, tag="T1")
                T2 = sp.tile([P, 2, sz], bf16, tag="T2")
                T3 = sp.tile([P, 2, sz], bf16, tag="T3")
                T4 = sp.tile([P, sz], bf16, tag="T4")
                S1 = sp.tile([P, 2, sz], bf16, tag="S1")
                S2 = sp.tile([P, 2, sz], bf16, tag="S2")
                Z1 = sp.tile([P, sz], bf16, tag="Z1")
                st[k].update(LPS=LPS, T4=T4, S2=S2, Z1=Z1, m1=m1)

                # --- decode (DVE tensor_scalar, 4x on 2-byte dtypes) ---
                # m1 bits: 0..8 = y0..y8 ; m2 bits: 0=y7, 4=y2, 5=y3, 6=y6
                # SGN: 0x8000 where label bit set (sign-flip masks for pp||ph)
                nc.vector.tensor_scalar(SGN[:], M2, 1, 15, Alu.bitwise_and, Alu.logical_shift_left)
                # MS: 0x8000 where s==0 (sign-flip for stroke: 1-ps arm)
                nc.vector.tensor_scalar(MS[:], m1, 0, 15, Alu.is_equal, Alu.logical_shift_left)
                nc.vector.tensor_scalar(G1[:], M2, 16, 0, Alu.bitwise_and, Alu.is_gt)
                nc.vector.tensor_scalar(G2[:], M2, 48, 32, Alu.bitwise_and, Alu.is_equal)
                nc.vector.tensor_scalar(G3[:, 0, :], m1, 48, 0, Alu.bitwise_and, Alu.is_equal)
                nc.vector.tensor_scalar(G3[:, 1, :], m2, 112, 64, Alu.bitwise_and, Alu.is_equal)
                nc.vector.tensor_scalar(GE[:], m2, 112, 0, Alu.bitwise_and, Alu.is_equal)

                # --- BCE folds: sign-flip (p-0.5) by label bit, then +0.5 ---
                nc.scalar.activation(EA[:], A2v, Act.Copy, bias=-0.5)
                nc.vector.tensor_tensor(XA[:], EA[:].bitcast(i16), SGN[:], op=Alu.bitwise_xor)
                nc.vector.tensor_scalar(PH[:], XA[:].bitcast(bf16), 0.5, 1.0, Alu.add, Alu.mult)
                nc.scalar.activation(ES[:], ps, Act.Copy, bias=-0.5)
                nc.vector.tensor_tensor(XS[:], ES[:].bitcast(i16), MS[:], op=Alu.bitwise_xor)
                # A1 += ln(ps_eff) ; ps_eff = (+-(ps-0.5)) + 0.5
                nc.scalar.activation(A1o[:], XS[:].bitcast(bf16), Act.Ln, bias=0.5,
                                     accum_out=accT[:, 3 * k : 3 * k + 1])
                nc.scalar.activation(LPS[:], ps, Act.Ln)

                # --- point/serve one-hot gated sums (Pool engine) ---
                nc.gpsimd.tensor_tensor(T1[:], G1[:], PQa, op=Alu.mult)
                nc.gpsimd.tensor_tensor(T2[:], G2[:], PQb, op=Alu.mult)
                nc.gpsimd.tensor_tensor(T3[:], G3[:], PQc, op=Alu.mult)
                nc.gpsimd.tensor_tensor(S1[:], T1[:], T2[:], op=Alu.add)
                nc.gpsimd.tensor_tensor(S2[:], S1[:], T3[:], op=Alu.add)
                # T4, Z1 on DVE (balance)
                nc.vector.tensor_tensor(T4[:], GE[:], Q3, op=Alu.mult)
                nc.vector.tensor_tensor(Z1[:], PH[:, 0, :], PH[:, 1, :], op=Alu.mult)

            def stage_c(k):
                off, sz = CHUNKS[k]
                s = st[k]
                SV = sp.tile([P, sz], bf16, tag="SV")
                Z2 = sp.tile([P, sz], bf16, tag="Z2")
                Z = sp.tile([P, sz], bf16, tag="Z")
                LZ = sp.tile([P, sz], bf16, tag="LZ")
                XPS = sp.tile([P, sz], bf16, tag="XPS")
                s.update(Z=Z, LZ=LZ)
                # A2 += s*ln(ps)  (gated sum of LPS)
                nc.vector._custom_dve(
                    SEL_GSUM, out=XPS[:], in0=s["LPS"][:], in1=s["m1"],
                    accum_out=accT[:, 3 * k + 1 : 3 * k + 2],
                )
                nc.gpsimd.tensor_tensor(SV[:], s["S2"][:, 1, :], s["T4"][:], op=Alu.add)
                nc.gpsimd.tensor_tensor(Z2[:], s["S2"][:, 0, :], SV[:], op=Alu.mult)
                nc.vector.tensor_tensor(Z[:], s["Z1"][:], Z2[:], op=Alu.mult)
                nc.scalar.activation(LZ[:], Z[:], Act.Ln)

            def stage_d(k):
                off, sz = CHUNKS[k]
                s = st[k]
                X3 = sp.tile([P, sz], bf16, tag="X3")
                # A3 += s*ln(Z)
                nc.vector._custom_dve(
                    SEL_GSUM, out=X3[:], in0=s["LZ"][:], in1=s["m1"],
                    accum_out=accT[:, 3 * k + 2 : 3 * k + 3],
                )

            # software pipeline: A(k) ... B(k) C(k-1) D(k-2)
            stage_a(0)
            stage_a(1)
            for k in range(K):
                if k + 2 < K:
                    stage_a(k + 2)
                if k >= 1:
                    stage_c(k - 1)
                if k >= 2:
                    stage_d(k - 2)
                stage_b(k)
            stage_c(K - 1)
            stage_d(K - 2)
            stage_d(K - 1)

            nc.sync.dma_start(acc_d[:], accT[:])

    return nc


_NC_CACHE = None


def _get_nc():
    global _NC_CACHE
    if _NC_CACHE is None:
        _NC_CACHE = _build_nc()
    return _NC_CACHE


def _to_bf16(x):
    import ml_dtypes

    return np.asarray(x, dtype=np.float32).astype(ml_dtypes.bfloat16)


def _pack_core(inputs, core):
    sl = slice(core * B_LOC, (core + 1) * B_LOC)
    planes = [
        inputs["y_pred_stroke"][sl, :, 0],
        inputs["y_pred_player"][sl, :, 0],
        inputs["y_pred_hand"][sl, :, 0],
        inputs["y_pred_point"][sl, :, 0],
        inputs["y_pred_point"][sl, :, 1],
        inputs["y_pred_point"][sl, :, 2],
        inputs["y_pred_serve"][sl, :, 0],
        inputs["y_pred_serve"][sl, :, 1],
        inputs["y_pred_serve"][sl, :, 2],
        inputs["y_pred_serve"][sl, :, 3],
    ]
    pl = np.empty((NPL, P, FD), dtype=np.uint16)
    for i, p in enumerate(planes):
        pl[i] = _to_bf16(p).reshape(P, FD).view(np.uint16)

    y = np.asarray(inputs["y_target"][sl], dtype=np.uint16)  # [B_LOC, S, 9] of 0/1
    yr = y.reshape(N, F)
    # m1: bits 0..8 = y0..y8 (9-bit bitmask; m1 != 0  <=>  s = any(y))
    w1 = (1 << np.arange(F, dtype=np.uint16)).astype(np.uint16)
    m1 = (yr * w1).sum(axis=1, dtype=np.uint16)
    # m2: bit re-placements for paired decode: 0=y7, 4=y2, 5=y3, 6=y6
    m2 = (
        yr[:, 7]
        | (yr[:, 2] << 4)
        | (yr[:, 3] << 5)
        | (yr[:, 6] << 6)
    ).astype(np.uint16)
    pl[10] = m1.reshape(P, FD)
    pl[11] = m2.reshape(P, FD)

    # chunk-major packed block: [chunk][p][plane][c] contiguous
    parts = []
    for off, sz in CHUNKS:
        parts.append(pl[:, :, off : off + sz].transpose(1, 0, 2).reshape(-1))
    blk = np.concatenate(parts).view(np.int16)
    return {"blk": blk}


def _shard_inputs(inputs):
    return [_pack_core(inputs, i) for i in range(NCORES)]


def kernel(**inputs) -> np.ndarray:
    nc = _get_nc()
    in_maps = _shard_inputs(inputs)
    res = run_bass_kernel_spmd(nc, in_maps, list(range(NCORES)))
    a1 = a2 = a3 = 0.0
    for r in res.results:
        a = r["acc"].astype(np.float64).reshape(P, K, 3)
        a1 += a[:, :, 0].sum()
        a2 += a[:, :, 1].sum()
        a3 += a[:, :, 2].sum()
    mean = -(W0 * a1 + C2 * a2 + a3) / float(B * S)
    return np.array([mean], dtype=np.float32)
